# revision 39
# baseline (speedup 1.0000x reference)
"""Trainium2 Bass kernel for nn_Dis_loss_69337952026648 (segment_reduce).

Strategy (fp8 DoubleRow + hybrid onehot):
  - Data-parallel over batch: 16 samples / 8 cores = 2 samples per core.
  - Per sample: 16-segment sums over 512x512 pixels of the 8 sim channels.
    Tag 0 (background) never contributes to the loss (present[0] is forced
    False in the reference), so only tags 1..16 are reduced -> M=16.
    Counts and masked counts are exact integers, computed on host via
    bincount (trivial vs. the 134MB of sim data).
  - Device: per 256-pixel chunk one PE matmul in DoubleRow perf mode
    (fp8e4m3, K=256 pixels per instruction): lhsT = onehot [128, 2, 16],
    rhs = sim values [128, 2, 8], accumulated into a [16, 8] fp32 PSUM
    tile per sample; 2048 matmuls per core, issued back-to-back at ~34ns
    (the PE instruction floor for this shape).
  - The onehot operand is produced two ways, interleaved per group, to
    balance DMA bytes against DVE cycles so neither stalls the PE:
      * ~half the groups: host-precomputed onehot, DMA'd (16B/pixel);
      * ~half + the small leading groups: generated on device by a DVE
        iota-compare from a 1B/pixel tag array loaded up front.
  - Host finishes the tiny 16x16 pairwise-distance loss in float32,
    mirroring the reference exactly.

Exactness notes: onehot values are 0/1 (exact in fp8) and PSUM accumulates
in fp32, so the discrete `present` mask matches the reference bit-exactly.
Only the sim segment sums carry fp8 rounding (~1e-5 on the final loss).
"""

import numpy as np

B, C, H, W = 16, 8, 512, 512
NSEG = 17
NTAG = 16  # tags 1..16 (tag 0 dropped)
NCORES = 8
SPC = B // NCORES  # samples per core
P = 128
PIX = H * W
NCHUNK2 = PIX // (2 * P)  # 1024 double-row chunks
NCH = C  # 8 sim channels (counts/masked-counts done on host via bincount)
LGG_VALUE = 3.0

_CACHE = {}


def _build_nc():
    """Build + compile the Bass module (cached)."""
    if "nc" in _CACHE:
        return _CACHE["nc"]
    import sys

    if "/opt/trn_rl_repo" not in sys.path:
        sys.path.append("/opt/trn_rl_repo")
    from contextlib import ExitStack

    from concourse import bacc, mybir, tile

    nc = bacc.Bacc("TRN2", target_bir_lowering=False, debug=False)
    v_in = nc.dram_tensor(
        "v", [SPC, P, NCHUNK2, 2, NCH], mybir.dt.float8e4, kind="ExternalInput"
    )
    oh_in = nc.dram_tensor(
        "oh", [SPC, P, NCHUNK2, 2, NTAG], mybir.dt.float8e4, kind="ExternalInput"
    )
    gk_in = nc.dram_tensor(
        "gk", [P, SPC, NCHUNK2, 2], mybir.dt.float8e4, kind="ExternalInput"
    )
    out = nc.dram_tensor(
        "o", [SPC, NTAG, 2, NCH], mybir.dt.float32, kind="ExternalOutput"
    )

    # Staged group sizes: small first groups so the first matmul's data
    # lands ASAP; bigger groups once the pipeline is warm.
    def group_sizes(s):
        if s == 0:
            gs = [32, 32, 64] + [128] * 7
        else:
            gs = [128] * 8
        assert sum(gs) == NCHUNK2
        return gs

    with tile.TileContext(nc) as tc:
        with ExitStack() as ctx:
            const = ctx.enter_context(tc.tile_pool(name="const", bufs=1))
            vpool = ctx.enter_context(tc.tile_pool(name="v", bufs=8))
            ohpool = ctx.enter_context(tc.tile_pool(name="oh", bufs=8))
            psum = ctx.enter_context(tc.tile_pool(name="ps", bufs=2, space="PSUM"))
            outpool = ctx.enter_context(tc.tile_pool(name="out", bufs=2))

            # iota (values 1..16) as fp8 for on-device onehot generation
            iota_i = const.tile([P, NTAG], mybir.dt.int32)
            nc.gpsimd.iota(iota_i[:], pattern=[[1, NTAG]], base=1, channel_multiplier=0)
            iota_f8 = const.tile([P, NTAG], mybir.dt.float8e4)
            nc.vector.tensor_copy(out=iota_f8[:], in_=iota_i[:])

            # all tags for the whole core in one tiny upfront DMA (4KB/partition)
            gk_all = const.tile([P, SPC, NCHUNK2, 2], mybir.dt.float8e4)
            nc.sync.dma_start(out=gk_all[:], in_=gk_in[:])

            gidx = 0
            for s in range(SPC):
                # two alternating PSUM accumulators so consecutive matmuls
                # hit different banks (avoids same-bank accumulate hazard)
                accs = [
                    psum.tile(
                        [NTAG, NCH],
                        mybir.dt.float32,
                        name=f"acc_{s}_{i}",
                        tag=f"acc{i}",
                    )
                    for i in range(2)
                ]
                k = 0
                gs = group_sizes(s)
                base = 0
                for g, gsize in enumerate(gs):
                    sl = slice(base, base + gsize)
                    base += gsize
                    gidx += 1
                    vt = vpool.tile([P, gsize, 2, NCH], mybir.dt.float8e4, tag="v")
                    nc.sync.dma_start(out=vt[:], in_=v_in[s, :, sl, :, :])
                    oht = ohpool.tile(
                        [P, gsize, 2, NTAG], mybir.dt.float8e4, tag="oh"
                    )
                    if (gsize < 128 or gidx % 2 == 0) and gidx > 2:
                        # on-device onehot via DVE iota-compare
                        nc.vector.tensor_tensor(
                            out=oht[:],
                            in0=gk_all[:, s, sl, :][:, :, :, None].to_broadcast(
                                [P, gsize, 2, NTAG]
                            ),
                            in1=iota_f8[:, None, None, :].to_broadcast(
                                [P, gsize, 2, NTAG]
                            ),
                            op=mybir.AluOpType.is_equal,
                        )
                    else:
                        nc.gpsimd.dma_start(out=oht[:], in_=oh_in[s, :, sl, :, :])
                    for j in range(gsize):
                        nc.tensor.matmul(
                            out=accs[k % 2][:],
                            lhsT=oht[:, j, :, :],
                            rhs=vt[:, j, :, :],
                            start=(k < 2),
                            stop=(k >= NCHUNK2 - 2),
                            perf_mode=mybir.MatmulPerfMode.DoubleRow,
                        )
                        k += 1
                ot = outpool.tile([NTAG, 2, NCH], mybir.dt.float32)
                nc.vector.tensor_copy(out=ot[:, 0, :], in_=accs[0][:])
                nc.vector.tensor_copy(out=ot[:, 1, :], in_=accs[1][:])
                nc.sync.dma_start(out=out[s], in_=ot[:])

    nc.compile()
    _CACHE["nc"] = nc
    return nc


def _pack_inputs(gt_kernel_key, training_mask, similarity_vector):
    """Host-side packing into per-core device input maps."""
    import ml_dtypes

    fp8 = ml_dtypes.float8_e4m3
    sim = np.asarray(similarity_vector, dtype=np.float32)
    gk = np.asarray(gt_kernel_key)
    tm = np.asarray(training_mask)

    # pixel q = j*256 + u*128 + p  ->  [b, p, j, u]
    # V[b, p, j, u, ch]
    V = np.ascontiguousarray(
        sim.reshape(B, C, NCHUNK2, 2, P).transpose(0, 4, 2, 3, 1)
    ).astype(fp8)

    # onehot over tags 1..16 via lookup table
    lut = np.zeros((NSEG, NTAG), dtype=fp8)
    for t in range(1, NSEG):
        lut[t, t - 1] = 1.0
    gkp = gk.reshape(B, NCHUNK2, 2, P).transpose(0, 3, 1, 2)  # [b, p, j, u]
    OH = lut[gkp]  # [b, p, j, u, 16]

    # exact integer counts on host (cheap): counts[t], masked_counts[t], t=1..16
    gk2 = gk.reshape(B, -1)
    mgk2 = (gk * tm).reshape(B, -1)
    counts = np.stack([np.bincount(g, minlength=NSEG)[1:NSEG] for g in gk2])
    masked = np.stack([np.bincount(g, minlength=NSEG)[1:NSEG] for g in mgk2])

    GK8 = gkp.astype(fp8)  # [b, p, j, u] tag values 0..16, exact in fp8

    in_maps = []
    for c in range(NCORES):
        sl = slice(c * SPC, (c + 1) * SPC)
        in_maps.append(
            {
                "v": np.ascontiguousarray(V[sl]),
                "oh": np.ascontiguousarray(OH[sl]),
                # partition-major [p, s, j, u] so the device loads it as one
                # contiguous run per partition
                "gk": np.ascontiguousarray(GK8[sl].transpose(1, 0, 2, 3)),
            }
        )
    return in_maps, counts.astype(np.float32), masked.astype(np.float32)


def _loss_from_stats(sums, counts, masked):
    """sums: [B, 16, 8] segment sums; counts/masked: [B, 16] -> scalar loss."""
    means = sums / np.maximum(counts, 1.0)[:, :, None]
    present = masked > 0  # [B, 16]
    diff = means[:, :, None, :] - means[:, None, :, :]
    dist = np.sqrt((diff * diff).sum(-1, dtype=np.float32) + np.float32(1e-12))
    pair = np.log(np.maximum(np.float32(LGG_VALUE) - dist, 0.0) ** 2 + 1.0)
    valid = present[:, :, None] & present[:, None, :] & ~np.eye(NTAG, dtype=bool)
    n_valid = valid.sum((1, 2)).astype(np.float32)
    losses = np.where(valid, pair, 0.0).sum((1, 2), dtype=np.float32) / np.maximum(
        n_valid, 1.0
    )
    sample_valid = (present.sum(1) >= 2).astype(np.float32)
    n = sample_valid.sum()
    total = (losses * sample_valid).sum(dtype=np.float32)
    out = total / max(n, np.float32(1.0)) if n > 0 else np.float32(0.0)
    return np.array(out, dtype=np.float32)


def _run_device(in_maps, trace=False, tmpdir=None):
    import sys

    if "/opt/trn_rl_repo" not in sys.path:
        sys.path.append("/opt/trn_rl_repo")
    from concourse.bass_utils import run_bass_kernel_spmd

    nc = _build_nc()
    kwargs = {}
    if trace:
        kwargs = {"trace": True, "tmpdir": tmpdir}
    return run_bass_kernel_spmd(nc, in_maps, core_ids=list(range(NCORES)), **kwargs)


def kernel(gt_kernel_key, training_mask, similarity_vector):
    in_maps, counts, masked = _pack_inputs(
        gt_kernel_key, training_mask, similarity_vector
    )
    res = _run_device(in_maps)
    sums = np.concatenate(
        [np.asarray(res.results[c]["o"], dtype=np.float32) for c in range(NCORES)],
        axis=0,
    ).sum(axis=2)  # merge the two per-sample PSUM accumulators
    return _loss_from_stats(sums, counts, masked)


# revision 43
# speedup vs baseline: 1.0300x; 1.0300x over previous
"""Trainium2 Bass kernel for nn_Dis_loss_69337952026648 (segment_reduce).

Strategy (fp8 DoubleRow + hybrid onehot):
  - Data-parallel over batch: 16 samples / 8 cores = 2 samples per core.
  - Per sample: 16-segment sums over 512x512 pixels of the 8 sim channels.
    Tag 0 (background) never contributes to the loss (present[0] is forced
    False in the reference), so only tags 1..16 are reduced -> M=16.
    Counts and masked counts are exact integers, computed on host via
    bincount (trivial vs. the 134MB of sim data).
  - Device: per 256-pixel chunk one PE matmul in DoubleRow perf mode
    (fp8e4m3, K=256 pixels per instruction): lhsT = onehot [128, 2, 16],
    rhs = sim values [128, 2, 8], accumulated into a [16, 8] fp32 PSUM
    tile per sample; 2048 matmuls per core, issued back-to-back at ~34ns
    (the PE instruction floor for this shape).
  - The onehot operand is produced two ways, interleaved per group, to
    balance DMA bytes against DVE cycles so neither stalls the PE:
      * ~half the groups: host-precomputed onehot, DMA'd (16B/pixel);
      * ~half + the small leading groups: generated on device by a DVE
        iota-compare from a 1B/pixel tag array loaded up front.
  - Host finishes the tiny 16x16 pairwise-distance loss in float32,
    mirroring the reference exactly.

Exactness notes: onehot values are 0/1 (exact in fp8) and PSUM accumulates
in fp32, so the discrete `present` mask matches the reference bit-exactly.
Only the sim segment sums carry fp8 rounding (~1e-5 on the final loss).
"""

import numpy as np

B, C, H, W = 16, 8, 512, 512
NSEG = 17
NTAG = 16  # tags 1..16 (tag 0 dropped)
NCORES = 8
SPC = B // NCORES  # samples per core
P = 128
PIX = H * W
NCHUNK2 = PIX // (2 * P)  # 1024 double-row chunks
NCH = C  # 8 sim channels (counts/masked-counts done on host via bincount)
LGG_VALUE = 3.0

_CACHE = {}


def _build_nc():
    """Build + compile the Bass module (cached)."""
    if "nc" in _CACHE:
        return _CACHE["nc"]
    import sys

    if "/opt/trn_rl_repo" not in sys.path:
        sys.path.append("/opt/trn_rl_repo")
    from contextlib import ExitStack

    from concourse import bacc, mybir, tile

    nc = bacc.Bacc("TRN2", target_bir_lowering=False, debug=False)
    v_in = nc.dram_tensor(
        "v", [SPC, P, NCHUNK2, 2, NCH], mybir.dt.float8e4, kind="ExternalInput"
    )
    oh_in = nc.dram_tensor(
        "oh", [SPC, P, NCHUNK2, 2, NTAG], mybir.dt.float8e4, kind="ExternalInput"
    )
    gk_in = nc.dram_tensor(
        "gk", [SPC, P, NCHUNK2, 2], mybir.dt.float8e4, kind="ExternalInput"
    )
    out = nc.dram_tensor(
        "o", [SPC, NTAG, 2, NCH], mybir.dt.float32, kind="ExternalOutput"
    )

    # Staged group sizes: small first groups so the first matmul's data
    # lands ASAP; bigger groups once the pipeline is warm.
    def group_sizes(s):
        if s == 0:
            gs = [32, 32, 64] + [128] * 7
        else:
            gs = [128] * 8
        assert sum(gs) == NCHUNK2
        return gs

    with tile.TileContext(nc) as tc:
        with ExitStack() as ctx:
            const = ctx.enter_context(tc.tile_pool(name="const", bufs=1))
            vpool = ctx.enter_context(tc.tile_pool(name="v", bufs=8))
            ohpool = ctx.enter_context(tc.tile_pool(name="oh", bufs=8))
            psum = ctx.enter_context(tc.tile_pool(name="ps", bufs=2, space="PSUM"))
            outpool = ctx.enter_context(tc.tile_pool(name="out", bufs=2))

            # iota (values 1..16) as fp8 for on-device onehot generation
            iota_i = const.tile([P, NTAG], mybir.dt.int32)
            nc.gpsimd.iota(iota_i[:], pattern=[[1, NTAG]], base=1, channel_multiplier=0)
            iota_f8 = const.tile([P, NTAG], mybir.dt.float8e4)
            nc.vector.tensor_copy(out=iota_f8[:], in_=iota_i[:])

            gk_tiles = {}
            gidx = 0
            for s in range(SPC):
                # two alternating PSUM accumulators so consecutive matmuls
                # hit different banks (avoids same-bank accumulate hazard)
                accs = [
                    psum.tile(
                        [NTAG, NCH],
                        mybir.dt.float32,
                        name=f"acc_{s}_{i}",
                        tag=f"acc{i}",
                    )
                    for i in range(2)
                ]
                k = 0
                gs = group_sizes(s)
                base = 0
                for g, gsize in enumerate(gs):
                    sl = slice(base, base + gsize)
                    base += gsize
                    gidx += 1
                    vt = vpool.tile([P, gsize, 2, NCH], mybir.dt.float8e4, tag="v")
                    nc.sync.dma_start(out=vt[:], in_=v_in[s, :, sl, :, :])
                    oht = ohpool.tile(
                        [P, gsize, 2, NTAG], mybir.dt.float8e4, tag="oh"
                    )
                    # sample-0 big groups are all DVE-generated (DVE works
                    # ahead off the tiny tag array, freeing the DMA ramp);
                    # sample-1 alternates DVE/DMA.
                    use_dve = (3 <= gidx <= 10) or (gidx >= 11 and gidx % 2 == 0)
                    if use_dve:
                        nc.vector.tensor_tensor(
                            out=oht[:],
                            in0=gk_tiles[s][:, sl, :][:, :, :, None].to_broadcast(
                                [P, gsize, 2, NTAG]
                            ),
                            in1=iota_f8[:, None, None, :].to_broadcast(
                                [P, gsize, 2, NTAG]
                            ),
                            op=mybir.AluOpType.is_equal,
                        )
                    else:
                        nc.gpsimd.dma_start(out=oht[:], in_=oh_in[s, :, sl, :, :])
                    if g == 0:
                        # per-sample tag array (2KB/partition), emitted after
                        # the first group's DMAs so it doesn't delay them
                        gkt = const.tile(
                            [P, NCHUNK2, 2],
                            mybir.dt.float8e4,
                            name=f"gk_s{s}",
                        )
                        nc.sync.dma_start(out=gkt[:], in_=gk_in[s])
                        gk_tiles[s] = gkt
                    for j in range(gsize):
                        nc.tensor.matmul(
                            out=accs[k % 2][:],
                            lhsT=oht[:, j, :, :],
                            rhs=vt[:, j, :, :],
                            start=(k < 2),
                            stop=(k >= NCHUNK2 - 2),
                            perf_mode=mybir.MatmulPerfMode.DoubleRow,
                        )
                        k += 1
                ot = outpool.tile([NTAG, 2, NCH], mybir.dt.float32)
                nc.vector.tensor_copy(out=ot[:, 0, :], in_=accs[0][:])
                nc.vector.tensor_copy(out=ot[:, 1, :], in_=accs[1][:])
                nc.sync.dma_start(out=out[s], in_=ot[:])

    nc.compile()
    _CACHE["nc"] = nc
    return nc


def _pack_inputs(gt_kernel_key, training_mask, similarity_vector):
    """Host-side packing into per-core device input maps."""
    import ml_dtypes

    fp8 = ml_dtypes.float8_e4m3
    sim = np.asarray(similarity_vector, dtype=np.float32)
    gk = np.asarray(gt_kernel_key)
    tm = np.asarray(training_mask)

    # pixel q = j*256 + u*128 + p  ->  [b, p, j, u]
    # V[b, p, j, u, ch]
    V = np.ascontiguousarray(
        sim.reshape(B, C, NCHUNK2, 2, P).transpose(0, 4, 2, 3, 1)
    ).astype(fp8)

    # onehot over tags 1..16 via lookup table
    lut = np.zeros((NSEG, NTAG), dtype=fp8)
    for t in range(1, NSEG):
        lut[t, t - 1] = 1.0
    gkp = gk.reshape(B, NCHUNK2, 2, P).transpose(0, 3, 1, 2)  # [b, p, j, u]
    OH = lut[gkp]  # [b, p, j, u, 16]

    # exact integer counts on host (cheap): counts[t], masked_counts[t], t=1..16
    gk2 = gk.reshape(B, -1)
    mgk2 = (gk * tm).reshape(B, -1)
    counts = np.stack([np.bincount(g, minlength=NSEG)[1:NSEG] for g in gk2])
    masked = np.stack([np.bincount(g, minlength=NSEG)[1:NSEG] for g in mgk2])

    GK8 = gkp.astype(fp8)  # [b, p, j, u] tag values 0..16, exact in fp8

    in_maps = []
    for c in range(NCORES):
        sl = slice(c * SPC, (c + 1) * SPC)
        in_maps.append(
            {
                "v": np.ascontiguousarray(V[sl]),
                "oh": np.ascontiguousarray(OH[sl]),
                "gk": np.ascontiguousarray(GK8[sl]),
            }
        )
    return in_maps, counts.astype(np.float32), masked.astype(np.float32)


def _loss_from_stats(sums, counts, masked):
    """sums: [B, 16, 8] segment sums; counts/masked: [B, 16] -> scalar loss."""
    means = sums / np.maximum(counts, 1.0)[:, :, None]
    present = masked > 0  # [B, 16]
    diff = means[:, :, None, :] - means[:, None, :, :]
    dist = np.sqrt((diff * diff).sum(-1, dtype=np.float32) + np.float32(1e-12))
    pair = np.log(np.maximum(np.float32(LGG_VALUE) - dist, 0.0) ** 2 + 1.0)
    valid = present[:, :, None] & present[:, None, :] & ~np.eye(NTAG, dtype=bool)
    n_valid = valid.sum((1, 2)).astype(np.float32)
    losses = np.where(valid, pair, 0.0).sum((1, 2), dtype=np.float32) / np.maximum(
        n_valid, 1.0
    )
    sample_valid = (present.sum(1) >= 2).astype(np.float32)
    n = sample_valid.sum()
    total = (losses * sample_valid).sum(dtype=np.float32)
    out = total / max(n, np.float32(1.0)) if n > 0 else np.float32(0.0)
    return np.array(out, dtype=np.float32)


def _run_device(in_maps, trace=False, tmpdir=None):
    import sys

    if "/opt/trn_rl_repo" not in sys.path:
        sys.path.append("/opt/trn_rl_repo")
    from concourse.bass_utils import run_bass_kernel_spmd

    nc = _build_nc()
    kwargs = {}
    if trace:
        kwargs = {"trace": True, "tmpdir": tmpdir}
    return run_bass_kernel_spmd(nc, in_maps, core_ids=list(range(NCORES)), **kwargs)


def kernel(gt_kernel_key, training_mask, similarity_vector):
    in_maps, counts, masked = _pack_inputs(
        gt_kernel_key, training_mask, similarity_vector
    )
    res = _run_device(in_maps)
    sums = np.concatenate(
        [np.asarray(res.results[c]["o"], dtype=np.float32) for c in range(NCORES)],
        axis=0,
    ).sum(axis=2)  # merge the two per-sample PSUM accumulators
    return _loss_from_stats(sums, counts, masked)
